# revision 1
# baseline (speedup 1.0000x reference)
"""BitExpert (BitNet-style MLP) Trainium2 kernel, 8-core data-parallel.

y = bitlinear(silu(bitlinear(x,w1)) * bitlinear(x,w3), w2)
  with per-token int8 activation quant and per-tensor ternary weight quant.

Strategy (8 NeuronCores, SPMD single NEFF):
 - Data-parallel over tokens: each core takes 1024 of 8192 token rows and a
   full copy of the weights in its own HBM.
 - Weights are passed host-pre-transposed (w1t = w1.T etc., a pure layout
   transform), so ternarized weight chunks stream from HBM directly in the
   [K-on-partitions] layout the PE needs: no on-device weight transposes.
 - Weight mean(|w|): each core abs-sums a distinct 1/8 slice per tensor;
   one tiny AllReduce (3 floats) combines them.
 - Quantized activations are ints in [-128,127]; ternary weights {-1,0,1};
   both exact in bf16, and f32 PSUM accumulation of the integer dot
   products (< 2^24) is exact -> all matmuls run at bf16 rate with exact
   integer arithmetic. Scales are applied on PSUM eviction.
 - round() == (v + 1.5*2^23) - 1.5*2^23  (round-half-even, == jnp.round).
 - x_q and h_q transposes use SBUF->SBUF DMA xbar transpose (bf16).
 - The pre-quant h (f32 [1024, 5632]) is staged through DRAM; its per-token
   absmax accumulates during phase 1; h_q^T stays SBUF-resident.
"""
import numpy as np

import concourse.mybir as mybir
import concourse.tile as tile
from concourse import bass_utils, bacc

F32 = mybir.dt.float32
BF16 = mybir.dt.bfloat16
AX = mybir.AxisListType
OP = mybir.AluOpType
ACTF = mybir.ActivationFunctionType

NCORES = 8
D = 2048           # d_model
H = 5632           # hidden
TOK = 8192         # total tokens
T = TOK // NCORES  # tokens per core (1024)
P = 128
TT = T // P        # token tiles per core (8)
HB = 512           # hidden block (phase 1)
NHB = H // HB      # 11
KD = D // P        # 16
KH = H // P        # 44
DB = 512           # d_model output block (phase 3)
NDB = D // DB      # 4
HQ = 1408          # phase-2 h chunk width
NHQ = H // HQ      # 4
KQ = HQ // P       # 11
XC = 1024          # x load chunk

MAGIC = 12582912.0             # 1.5 * 2^23
EPS = 1e-5
NW = float(H * D)
WSLF = (H * D) // NCORES // P  # 11264
RECIP_NW = float(np.float32(1.0) / np.float32(NW))


def _build(use_collective=True):
    nc = bacc.Bacc("TRN2", target_bir_lowering=False, debug=False,
                   num_devices=NCORES if use_collective else 1)
    x = nc.dram_tensor("x", [T, D], F32, kind="ExternalInput").ap()
    w1t = nc.dram_tensor("w1t", [D, H], F32, kind="ExternalInput").ap()
    w2t = nc.dram_tensor("w2t", [H, D], F32, kind="ExternalInput").ap()
    w3t = nc.dram_tensor("w3t", [D, H], F32, kind="ExternalInput").ap()
    wsl = nc.dram_tensor("wsl", [3, P, WSLF], F32, kind="ExternalInput").ap()
    y = nc.dram_tensor("y", [T, D], F32, kind="ExternalOutput").ap()

    cc_in = nc.dram_tensor("cc_in", [1, 4], F32)
    cc_out = nc.dram_tensor("cc_out", [1, 4], F32, addr_space="Shared")

    with tile.TileContext(nc) as tc:
        _body(nc, tc, x, w1t, w2t, w3t, wsl, y, cc_in, cc_out,
              use_collective)
    nc.compile()
    return nc


def _body(nc, tc, x, w1t, w2t, w3t, wsl, y, cc_in, cc_out,
          use_collective=True):
    ctxs = []

    def pool(name, bufs, space="SBUF"):
        cm = tc.tile_pool(name=name, bufs=bufs, space=space)
        p = cm.__enter__()
        ctxs.append(cm)
        return p

    singles = pool("singles", 1)
    dramp = pool("dram", 1, space="DRAM")
    actT = pool("actT", 1)     # xqT then hqT (sequential lifetimes)
    wload = pool("wload", 4)   # [P, 512] f32 weight chunks
    wc = pool("wc", 8)         # [P, 512] bf16 ternary chunks (phase 3)
    gload = pool("gload", 5)   # [P, HQ] f32: x chunks, wsl chunks, h chunks
    qb = pool("qb", 3)         # bf16 quantized naturals (x and h chunks)
    wT = pool("wT", 3)         # [P, KD, HB] bf16 ternary w1/w3 blocks
    hwork = pool("hwork", 2)   # eviction tiles + small scalars
    sApool = pool("sApool", 2)
    yout = pool("yout", 2)
    hq0p = pool("hq0p", 1)
    psum = pool("psum", 8, space="PSUM")

    g_dram = dramp.tile([T, H], F32)

    # persistent per-token scalars (one column per token tile)
    mh_all = singles.tile([P, TT], F32)
    sx_all = singles.tile([P, TT], F32)
    al_all = singles.tile([P, TT], F32)
    be_all = singles.tile([P, TT], F32)
    sh_all = singles.tile([P, TT], F32)
    de_all = singles.tile([P, TT], F32)
    rx_all = singles.tile([P, TT], F32)
    cvec = singles.tile([P, 8], F32)   # [c1 c3 c2 _ s1 s3 s2 _]
    negmagic = singles.tile([P, 1], F32)
    nc.vector.memset(negmagic[:], -MAGIC)
    nc.vector.memset(mh_all[:], 0.0)

    # ---------------- x: absmax, quantize, transpose ----------------
    xqT = actT.tile([P, KD, T], BF16, tag="actT")

    def emit_x_tile(tt):
        xts = []
        mx = hwork.tile([P, 1], F32, tag="mx")
        for cix in range(D // XC):
            xt = gload.tile([P, XC], F32, tag="gld", name=f"xt{cix}")
            nc.sync.dma_start(
                xt[:], x[tt * P:(tt + 1) * P, cix * XC:(cix + 1) * XC])
            xts.append(xt)
            mc = hwork.tile([P, 1], F32, tag="mxc")
            nc.vector.tensor_reduce(mc[:], xt[:], AX.X, OP.max,
                                    apply_absolute_value=True)
            if cix == 0:
                nc.vector.tensor_scalar(mx[:], mc[:], EPS, None, OP.max)
            else:
                nc.vector.tensor_tensor(mx[:], mx[:], mc[:], OP.max)
        rec = hwork.tile([P, 1], F32, tag="rec")
        nc.vector.reciprocal(rec[:], mx[:])
        sx = sx_all[:, tt:tt + 1]
        nc.vector.tensor_scalar(sx, rec[:], 127.0, None, OP.mult)
        nc.vector.reciprocal(rx_all[:, tt:tt + 1], sx)
        for cix in range(D // XC):
            xt = xts[cix]
            nc.vector.tensor_scalar(xt[:], xt[:], sx, MAGIC, OP.mult, OP.add)
            xq = qb.tile([P, XC], BF16, tag="qb")
            nc.vector.tensor_scalar(xq[:], xt[:], MAGIC, None, OP.subtract)
            nc.sync.dma_start_transpose(
                xqT[:, cix * (XC // P):(cix + 1) * (XC // P),
                    tt * P:(tt + 1) * P], xq[:])


    # ---------------- weight mean partials + AllReduce ----------------
    partials = singles.tile([P, 4], F32)
    nc.vector.memset(partials[:], 1.0)  # col 3 dummy (avoids 1/0 later)
    x_emitted = [0]
    for j in range(3):
        for ch in range(8):
            wchunk = gload.tile([P, HQ], F32, tag="gld")
            nc.sync.dma_start(wchunk[:], wsl[j, :, ch * HQ:(ch + 1) * HQ])
            s1 = hwork.tile([P, KQ], F32, tag="msum1")
            nc.vector.tensor_reduce(
                s1[:], wchunk[:].rearrange("p (a b) -> p a b", b=P),
                AX.X, OP.add, apply_absolute_value=True)
            s2 = hwork.tile([P, 1], F32, tag="msum2")
            nc.vector.tensor_reduce(s2[:], s1[:], AX.X, OP.add)
            if ch == 0:
                nc.vector.tensor_copy(partials[:, j:j + 1], s2[:])
            else:
                nc.vector.tensor_tensor(partials[:, j:j + 1],
                                        partials[:, j:j + 1], s2[:], OP.add)
            if ch % 3 == 2 and x_emitted[0] < TT:
                emit_x_tile(x_emitted[0])
                x_emitted[0] += 1
    ones = singles.tile([P, 1], F32)
    nc.vector.memset(ones[:], 1.0)
    psums = psum.tile([1, 4], F32, tag="ps")
    nc.tensor.matmul(psums[:], ones[:], partials[:], start=True, stop=True)
    row4 = singles.tile([1, 4], F32)
    nc.vector.tensor_copy(row4[:], psums[:])
    nc.sync.dma_start(cc_in.ap(), row4[:])
    if use_collective:
        nc.gpsimd.collective_compute(
            "AllReduce", OP.add, replica_groups=[list(range(NCORES))],
            ins=[cc_in.ap()], outs=[cc_out.ap()])
    else:
        nc.sync.dma_start(cc_out.ap(), row4[:])
    sums = singles.tile([1, 4], F32)
    nc.sync.dma_start(sums[:], cc_out.ap())
    row8 = singles.tile([1, 8], F32)
    nc.vector.tensor_scalar(row8[:, 0:4], sums[:], RECIP_NW, EPS,
                            OP.mult, OP.max)
    nc.vector.reciprocal(row8[:, 4:8], row8[:, 0:4])
    nc.gpsimd.partition_broadcast(cvec[:], row8[:])
    c1, c3, c2 = cvec[:, 0:1], cvec[:, 1:2], cvec[:, 2:3]
    s1c, s3c, s2c = cvec[:, 4:5], cvec[:, 5:6], cvec[:, 6:7]

    for tt in range(x_emitted[0], TT):
        emit_x_tile(tt)
    nc.vector.tensor_tensor(al_all[:], rx_all[:], c1.to_broadcast((P, TT)),
                            OP.mult)
    nc.vector.tensor_tensor(be_all[:], rx_all[:], c3.to_broadcast((P, TT)),
                            OP.mult)

    def tern_chunk(wt_ap, r0, col0, ncols, scol, out_ap):
        """[128, ncols] rows of pre-transposed weight -> ternary bf16."""
        wf = wload.tile([P, ncols], F32, tag="wf")
        nc.sync.dma_start(wf[:], wt_ap[r0:r0 + P, col0:col0 + ncols])
        nc.vector.tensor_scalar(wf[:], wf[:], scol, 1.49, OP.mult, OP.min)
        nc.vector.tensor_scalar(wf[:], wf[:], -1.49, MAGIC, OP.max, OP.add)
        nc.scalar.activation(out_ap, wf[:], ACTF.Identity,
                             bias=negmagic[:, 0:1])

    def build_wts(hb):
        """Ternary [d-on-partitions] blocks of w1,w3 for hidden block hb."""
        wTs = []
        for wt_ap, scol in ((w1t, s1c), (w3t, s3c)):
            dst = wT.tile([P, KD, HB], BF16, tag="wT")
            for ko in range(KD):
                tern_chunk(wt_ap, ko * P, hb * HB, HB, scol, dst[:, ko, :])
            wTs.append(dst)
        return wTs

    shtmp = singles.tile([P, TT], F32)
    rh = singles.tile([P, TT], F32)
    hq_q0 = hq0p.tile([P, KQ, T], BF16)

    def emit_sh(tt):
        cs = slice(tt, tt + 1)
        nc.vector.tensor_scalar(shtmp[:, cs], mh_all[:, cs], EPS, None,
                                OP.max)
        nc.vector.reciprocal(shtmp[:, cs], shtmp[:, cs])
        nc.vector.tensor_scalar(sh_all[:, cs], shtmp[:, cs], 127.0, None,
                                OP.mult)
        nc.vector.reciprocal(rh[:, cs], sh_all[:, cs])
        nc.vector.tensor_tensor(de_all[:, cs], rh[:, cs], c2, OP.mult)

    def emit_hq_chunk(q, tt, dst_ap):
        gt = gload.tile([P, HQ], F32, tag="gld")
        nc.sync.dma_start(gt[:], g_dram[tt * P:(tt + 1) * P,
                                        q * HQ:(q + 1) * HQ])
        nc.vector.tensor_scalar(gt[:], gt[:], sh_all[:, tt:tt + 1],
                                MAGIC, OP.mult, OP.add)
        hqn = qb.tile([P, HQ], BF16, tag="qb")
        nc.vector.tensor_scalar(hqn[:], gt[:], MAGIC, None, OP.subtract)
        nc.sync.dma_start_transpose(dst_ap, hqn[:])

    # ---------------- phase 1: mm1/mm3, h = silu(h1)*h3 ----------------
    # Weight builds are emitted one block ahead so their DVE/ACT passes are
    # not head-of-line blocked behind this block's PSUM-eviction waits.
    pending = [build_wts(0)]
    for hb in range(NHB):
        if hb + 1 < NHB:
            pending.append(build_wts(hb + 1))
        wTs = pending[hb]
        for tt in range(TT):
            tsl = slice(tt * P, (tt + 1) * P)
            psa = psum.tile([P, HB], F32, tag="ps")
            for ko in range(KD):
                nc.tensor.matmul(psa[:], xqT[:, ko, tsl], wTs[0][:, ko, :],
                                 start=(ko == 0), stop=(ko == KD - 1))
            psb = psum.tile([P, HB], F32, tag="ps")
            for ko in range(KD):
                nc.tensor.matmul(psb[:], xqT[:, ko, tsl], wTs[1][:, ko, :],
                                 start=(ko == 0), stop=(ko == KD - 1))
            sA = sApool.tile([P, HB], F32, tag="sA")
            nc.scalar.activation(sA[:], psa[:], ACTF.Silu,
                                 scale=al_all[:, tt:tt + 1])
            h3 = hwork.tile([P, HB], F32, tag="h3")
            nc.scalar.mul(h3[:], psb[:], be_all[:, tt:tt + 1])
            hh = hwork.tile([P, HB], F32, tag="hh")
            nc.vector.tensor_tensor(hh[:], sA[:], h3[:], OP.mult)
            mpart = hwork.tile([P, 1], F32, tag="mpart")
            nc.vector.tensor_reduce(mpart[:], hh[:], AX.X, OP.max,
                                    apply_absolute_value=True)
            nc.vector.tensor_tensor(mh_all[:, tt:tt + 1],
                                    mh_all[:, tt:tt + 1], mpart[:], OP.max)
            nc.sync.dma_start(g_dram[tsl, hb * HB:(hb + 1) * HB], hh[:])
            if hb == NHB - 1:
                emit_sh(tt)
                emit_hq_chunk(0, tt, hq_q0[:, :, tt * P:(tt + 1) * P])


    # ---------------- phase 2: quantize h, transpose ----------------
    # db=0 w2 ternary chunks are emitted interleaved with the h-quant bands
    # so neither blocks the other in the DVE stream.
    hqT = actT.tile([P, KH - KQ, T], BF16, tag="actT")
    wq0 = {}
    for hc in range(0, KQ):
        wq = wc.tile([P, DB], BF16, tag="wc", name=f"wq0_{hc}")
        tern_chunk(w2t, hc * P, 0, DB, s2c, wq[:])
        wq0[hc] = wq
    for q in range(1, NHQ):
        for tt in range(TT):
            emit_hq_chunk(
                q, tt,
                hqT[:, (q - 1) * KQ:q * KQ, tt * P:(tt + 1) * P])
        for hc in range(q * KQ, (q + 1) * KQ):
            wq = wc.tile([P, DB], BF16, tag="wc", name=f"wq0_{hc}")
            tern_chunk(w2t, hc * P, 0, DB, s2c, wq[:])
            wq0[hc] = wq

    # ---------------- phase 3: mm2, scale, store ----------------
    # ternary chunks for block db+1 are emitted before block db's PSUM
    # eviction waits so the DVE/ACT streams are not head-of-line blocked.
    wq_pend = {hc: wq0[hc] for hc in range(KH)}
    for db in range(NDB):
        psys = [psum.tile([P, DB], F32, tag="ps", name=f"psy{i}")
                for i in range(TT)]
        wq_cur, wq_pend = wq_pend, {}
        for hc in range(KH):
            for tt in range(TT):
                if hc < KQ:
                    lhsT = hq_q0[:, hc, tt * P:(tt + 1) * P]
                else:
                    lhsT = hqT[:, hc - KQ, tt * P:(tt + 1) * P]
                nc.tensor.matmul(psys[tt][:], lhsT, wq_cur[hc][:],
                                 start=(hc == 0), stop=(hc == KH - 1))
            if db + 1 < NDB and hc % 3 == 2:
                for hc2 in range(hc - 2, hc + 1):
                    wq = wc.tile([P, DB], BF16, tag="wc")
                    tern_chunk(w2t, hc2 * P, (db + 1) * DB, DB, s2c, wq[:])
                    wq_pend[hc2] = wq
        if db + 1 < NDB:
            for hc2 in range(42, KH):
                wq = wc.tile([P, DB], BF16, tag="wc")
                tern_chunk(w2t, hc2 * P, (db + 1) * DB, DB, s2c, wq[:])
                wq_pend[hc2] = wq
        for tt in range(TT):
            ysb = yout.tile([P, DB], F32)
            nc.vector.tensor_scalar(ysb[:], psys[tt][:],
                                    de_all[:, tt:tt + 1], None, OP.mult)
            nc.sync.dma_start(y[tt * P:(tt + 1) * P,
                                db * DB:(db + 1) * DB], ysb[:])

    for cm in reversed(ctxs):
        cm.__exit__(None, None, None)


_NC_CACHE = None


def _get_nc():
    global _NC_CACHE
    if _NC_CACHE is None:
        _NC_CACHE = _build()
    return _NC_CACHE


def kernel(x, w1, w2, w3, trace=False):
    x = np.ascontiguousarray(np.asarray(x, dtype=np.float32))
    w1 = np.asarray(w1, dtype=np.float32)
    w2 = np.asarray(w2, dtype=np.float32)
    w3 = np.asarray(w3, dtype=np.float32)
    w1t = np.ascontiguousarray(w1.T)
    w2t = np.ascontiguousarray(w2.T)
    w3t = np.ascontiguousarray(w3.T)
    B, S, Dm = x.shape
    xf = x.reshape(B * S, Dm)

    in_maps = []
    slc = (H * D) // NCORES
    for i in range(NCORES):
        wsl = np.stack([
            w1.reshape(-1)[i * slc:(i + 1) * slc].reshape(P, WSLF),
            w3.reshape(-1)[i * slc:(i + 1) * slc].reshape(P, WSLF),
            w2.reshape(-1)[i * slc:(i + 1) * slc].reshape(P, WSLF),
        ]).astype(np.float32)
        in_maps.append(dict(
            x=np.ascontiguousarray(xf[i * T:(i + 1) * T]),
            w1t=w1t, w2t=w2t, w3t=w3t, wsl=wsl))

    nc = _get_nc()
    res = bass_utils.run_bass_kernel_spmd(
        nc, in_maps, core_ids=list(range(NCORES)),
        trace=trace, trace_cores=[0] if trace else None)
    out = np.concatenate([res.results[i]["y"] for i in range(NCORES)], axis=0)
    if trace:
        kernel.last_results = res
    return out.reshape(B, S, Dm)

